# revision 22
# baseline (speedup 1.0000x reference)
"""Binary (sign-quantized weight) 3x3 conv, stride 1, pad 1, on 8 trn2 cores.

Problem: x[32,128,56,56] f32, weight[256,128,3,3] f32, bias[256] f32
         y = conv2d(x, sign(weight), pad=1) + bias      -> [32,256,56,56] f32

Strategy (v4, fp8 DoubleRow + per-block chunk tiles):
  - Data-parallel over batch: 4 images per core, weight/bias replicated.
  - x is decomposed into two fp8e4 planes: hi = e4m3(x), lo = e4m3(x-hi).
    One DoubleRow matmul contracts both planes (K = 2x128) against a
    (sign(w), sign(w)) stationary pair at 0.5 cycles/output element --
    2x the bf16 rate at ~bf16 accuracy.
  - Each 8-row output block owns a small chunk tile [128, 2, 572]
    holding its 10 input rows (block + halo) flat at physical width 57:
    one zero pad col per row, shared between row r's right edge and row
    r+1's left edge. A 3x3 tap is then ONE flat shifted-segment matmul
    (junk output col cp=0 per row discarded at drain; vertical padding =
    row-range narrowing, PSUM has_written covers partial writes). Small
    per-block tiles keep Tile's bounding-range dependencies tight so
    matmuls start as soon as their own chunk is packed.
  - DMA: inputs on the Pool queue, weights (split per kh) + outputs on
    the SP queue; img 0 arrives in 7 halo-straddling chunks alternating
    between queues, imgs 1-3 as single transfers. Output staged bf16
    per (image, co-half) and DMA'd in halves (finer on the last image);
    host upcasts to f32.
  - Drains (bias add, PSUM -> SBUF bf16) alternate DVE/ACT; hi-packs on
    ACT, lo-packs alternate DVE/gpsimd.
  - A stream of small zero matmuls bridges the PE from t~0 to the first
    real matmul so the p-state ramp never resets.
"""

import sys

sys.path.insert(0, "/opt/trn_rl_repo")

from contextlib import ExitStack

import numpy as np

B, CI, CO, KK, H, W = 32, 128, 256, 3, 56, 56
N_CORES = 8
B_SH = B // N_CORES  # 4 images per core
WP = W + 1  # physical row width: shared zero pad col
ROWS = 8  # output rows per block
N_RB = H // ROWS  # 7 row blocks
NOUT = ROWS * WP  # 456 <= 512 (one PSUM bank)
SLOTS = ROWS + 2  # input rows per chunk tile (block + halo)
LC = 1 + SLOTS * WP + 1  # chunk tile flat length (front/back guards)
XPC_BUFS = 6

_NC_CACHE = None


def _build():
    import concourse.tile as tile
    from concourse import bacc, mybir

    nc = bacc.Bacc("TRN2", target_bir_lowering=False, debug=False)

    x_d = nc.dram_tensor("x", [B_SH, CI, H, W], mybir.dt.float32, kind="ExternalInput")
    wt_d = nc.dram_tensor(
        "wt", [CI, KK * KK * 2 * CO], mybir.dt.float8e4, kind="ExternalInput"
    )
    b_d = nc.dram_tensor("bias2", [128, CO // 128], mybir.dt.float32, kind="ExternalInput")
    y_d = nc.dram_tensor("y", [B_SH, CO, H * W], mybir.dt.bfloat16, kind="ExternalOutput")

    x_img = x_d.ap().rearrange("b c h w -> b c (h w)")
    y_ap = y_d.ap()

    with tile.TileContext(nc) as tc:
        with ExitStack() as ctx:
            singles = ctx.enter_context(tc.tile_pool(name="singles", bufs=1))
            xpc_pool = ctx.enter_context(tc.tile_pool(name="xpc", bufs=XPC_BUFS))
            ps_pool = ctx.enter_context(
                tc.tile_pool(name="ps", bufs=8, space="PSUM")
            )
            yo_pool = ctx.enter_context(tc.tile_pool(name="yo", bufs=2))

            # [ci, tap, (A,B) slab pair, co] fp8; A and B both sign(w)
            w_bin = singles.tile([CI, KK * KK, 2, CO], mybir.dt.float8e4)
            wt_ap = wt_d.ap().rearrange("p (t two c) -> p t two c", two=2, c=CO)

            # PE warm-up bridge
            N_WARM = 45
            warm_w = singles.tile([128, 128], mybir.dt.bfloat16)
            warm_x = singles.tile([128, 112], mybir.dt.bfloat16)
            nc.vector.memset(warm_w[:, :], 0.0)
            nc.vector.memset(warm_x[:, :], 0.0)
            for _ in range(N_WARM):
                warm_ps = ps_pool.tile([128, 112], mybir.dt.float32, tag="ps")
                nc.tensor.matmul(
                    warm_ps[:, :], warm_w[:, :], warm_x[:, :], start=True, stop=True
                )
            warm_a = singles.tile([128, 1], mybir.dt.float32)
            nc.vector.memset(warm_a[:, :], 0.0)
            nc.scalar.activation(
                warm_a[:, :], warm_a[:, :], mybir.ActivationFunctionType.Identity
            )

            # whole-image f32 staging, double buffered (static so reuse keeps
            # SBUF offsets fixed)
            xfs = [
                singles.tile([CI, H * W], mybir.dt.float32, name=f"xf{i}")
                for i in range(2)
            ]

            n_alloc = 0

            def alloc_chunk():
                """Chunk tile; zero the pad cols once per physical buffer.

                The pool hands buffers out round-robin, packs never write pad
                cols, so zeroing the first XPC_BUFS allocations covers all."""
                nonlocal n_alloc
                t = xpc_pool.tile([CI, 2, LC], mybir.dt.float8e4, tag="xpc")
                if n_alloc < XPC_BUFS:
                    for i in range(2):
                        nc.gpsimd.memset(
                            t[:, i, 1 : 1 + SLOTS * WP].rearrange(
                                "p (h w) -> p h w", w=WP
                            )[:, :, 0:1],
                            0.0,
                        )
                        # back guard doubles as slot 9's right pad
                        nc.gpsimd.memset(t[:, i, LC - 1 : LC], 0.0)
                n_alloc += 1
                return t

            def pack_chunk(xpt, xf, rb, lo_eng, sub=None):
                """hi/lo packs of block rb's input rows into its chunk tile."""
                r0 = rb * ROWS
                ra = max(r0 - 1, 0)  # first source row
                rz = min(r0 + ROWS + 1, H)  # end source row
                if sub is not None:
                    ra, rz = sub
                s0 = ra - (r0 - 1)  # first slot written
                n = rz - ra
                body = lambda i: xpt[
                    :, i, 1 + s0 * WP : 1 + (s0 + n) * WP
                ].rearrange("p (h w) -> p h w", w=WP)[:, :, 1 : 1 + W]
                xfv = xf.rearrange("p (h w) -> p h w", w=W)[:, ra:rz, :]
                hi = body(0)
                nc.scalar.activation(
                    hi, xfv, mybir.ActivationFunctionType.Identity
                )
                lo_eng.tensor_tensor(
                    out=body(1), in0=xfv, in1=hi, op=mybir.AluOpType.subtract
                )

            def wt_dma(kh):
                nc.sync.dma_start(
                    out=w_bin[:, kh * KK : (kh + 1) * KK, :, :],
                    in_=wt_ap[:, kh * KK : (kh + 1) * KK, :, :],
                )

            # startup: img-0 chunk DMAs straddle block halos and alternate
            # Pool/SP queues; weight thirds interleave on SP; bias last
            bias_sb = singles.tile([128, CO // 128], mybir.dt.float32)
            bounds = [0, 9, 17, 25, 33, 41, 49, 56]
            chunk_engs = [
                nc.sync, nc.gpsimd, nc.sync, nc.gpsimd,
                nc.sync, nc.gpsimd, nc.sync,
            ]
            xpcs = {}

            def lo_eng_for(rb):
                return nc.gpsimd if rb % 3 == 2 else nc.vector

            def chunk0_dma(c):
                r0, r1 = bounds[c], bounds[c + 1]
                chunk_engs[c].dma_start(
                    out=xfs[0][:, r0 * W : r1 * W], in_=x_img[0, :, r0 * W : r1 * W]
                )
                xpcs[0, c] = alloc_chunk()
                pack_chunk(xpcs[0, c], xfs[0], c, lo_eng_for(c))

            # chunk 0 rides both queues in halves so its packs start earliest
            nc.sync.dma_start(out=xfs[0][:, 0 : 5 * W], in_=x_img[0, :, 0 : 5 * W])
            nc.gpsimd.dma_start(
                out=xfs[0][:, 5 * W : 9 * W], in_=x_img[0, :, 5 * W : 9 * W]
            )
            xpcs[0, 0] = alloc_chunk()
            pack_chunk(xpcs[0, 0], xfs[0], 0, nc.vector, sub=(0, 5))
            pack_chunk(xpcs[0, 0], xfs[0], 0, nc.vector, sub=(5, 9))
            wt_dma(0)
            chunk0_dma(1)
            chunk0_dma(2)
            wt_dma(1)
            chunk0_dma(3)
            wt_dma(2)
            chunk0_dma(4)
            chunk0_dma(5)
            chunk0_dma(6)
            nc.sync.dma_start(out=bias_sb[:, :], in_=b_d.ap())

            for b in range(B_SH):
                if b > 0:
                    xf = xfs[b % 2]
                    nc.gpsimd.dma_start(out=xf[:, :], in_=x_img[b, :, :])
                    for rb in range(N_RB):
                        xpcs[b, rb] = alloc_chunk()
                        pack_chunk(xpcs[b, rb], xf, rb, lo_eng_for(rb))

                yb = yo_pool.tile(
                    [128, CO // 128, H * W], mybir.dt.bfloat16, tag="yb"
                )
                for rb in range(N_RB):
                    r0 = rb * ROWS
                    xpt = xpcs.pop((b, rb))
                    for c2 in range(CO // 128):
                        ps = ps_pool.tile([128, NOUT], mybir.dt.float32, tag="ps")
                        i = 0
                        for kh in range(KK):
                            # rows (within block) whose input row is in [0, H)
                            a = max(0, (1 - kh) - r0)
                            bb = min(ROWS, (H + 1) - kh - r0)
                            for kw in range(KK):
                                q0 = 1 + (a + kh) * WP + (kw - 1)
                                nrow = bb - a
                                assert 0 <= q0 and q0 + nrow * WP <= LC
                                nc.tensor.matmul(
                                    ps[:, a * WP : bb * WP],
                                    w_bin[:, kh * KK + kw, :, c2 * 128 : (c2 + 1) * 128],
                                    xpt[:, :, q0 : q0 + nrow * WP],
                                    start=(i == 0),
                                    stop=(i == KK * KK - 1),
                                    perf_mode=mybir.MatmulPerfMode.DoubleRow,
                                    skip_group_check=True,
                                )
                                i += 1
                        ys = yb[:, c2, rb * ROWS * W : (rb + 1) * ROWS * W]
                        ps3 = ps.rearrange("p (r w) -> p r w", w=WP)
                        if (rb + c2) % 2 == 0:
                            nc.vector.tensor_scalar_add(
                                ys.rearrange("p (r w) -> p r w", w=W),
                                ps3[:, :, 1 : 1 + W],
                                bias_sb[:, c2 : c2 + 1],
                            )
                        else:
                            nc.scalar.activation(
                                ys.rearrange("p (r w) -> p r w", w=W),
                                ps3[:, :, 1 : 1 + W],
                                mybir.ActivationFunctionType.Identity,
                                bias=bias_sb[:, c2 : c2 + 1],
                            )
                    # output DMAs: rb 0-3 as one transfer per (b,c2); the rest
                    # per-rb on the last image (short tail) or as one transfer
                    if rb == 3:
                        for c2 in range(CO // 128):
                            nc.sync.dma_start(
                                out=y_ap[b, c2 * 128 : (c2 + 1) * 128, : 4 * ROWS * W],
                                in_=yb[:, c2, : 4 * ROWS * W],
                            )
                    elif rb == N_RB - 1:
                        if b == B_SH - 1:
                            # keep the final (latest-draining) transfers on the
                            # fast HWDGE queue; push earlier ones to Pool
                            for r2 in range(4, N_RB):
                                for c2 in range(CO // 128):
                                    eng = nc.gpsimd if (c2 == 1 and r2 < 6) else nc.sync
                                    eng.dma_start(
                                        out=y_ap[
                                            b,
                                            c2 * 128 : (c2 + 1) * 128,
                                            r2 * ROWS * W : (r2 + 1) * ROWS * W,
                                        ],
                                        in_=yb[:, c2, r2 * ROWS * W : (r2 + 1) * ROWS * W],
                                    )
                        else:
                            for c2 in range(CO // 128):
                                nc.sync.dma_start(
                                    out=y_ap[b, c2 * 128 : (c2 + 1) * 128, 4 * ROWS * W :],
                                    in_=yb[:, c2, 4 * ROWS * W :],
                                )
    nc.compile()
    return nc


def _get_nc():
    global _NC_CACHE
    if _NC_CACHE is None:
        _NC_CACHE = _build()
    return _NC_CACHE


def kernel(x, weight, bias):
    from concourse.bass_utils import run_bass_kernel_spmd

    import ml_dtypes

    x = np.ascontiguousarray(np.asarray(x, dtype=np.float32))
    weight = np.asarray(weight, dtype=np.float32)
    bias = np.asarray(bias, dtype=np.float32)

    # binarize on host; {-1,0,1} is exact in fp8e4. [co,ci,kh,kw] ->
    # [ci, (kh kw), co], duplicated into (A,B) slab pairs for DoubleRow.
    ws = np.sign(weight).transpose(1, 2, 3, 0).reshape(CI, KK * KK, 1, CO)
    wt = np.ascontiguousarray(
        np.broadcast_to(ws, (CI, KK * KK, 2, CO)).reshape(CI, KK * KK * 2 * CO)
    ).astype(ml_dtypes.float8_e4m3fn)
    # bias2[p, c2] = bias[c2*128 + p]
    bias2 = np.ascontiguousarray(bias.reshape(CO // 128, 128).T)

    nc = _get_nc()
    in_maps = [
        {"x": x[i * B_SH : (i + 1) * B_SH], "wt": wt, "bias2": bias2}
        for i in range(N_CORES)
    ]
    res = run_bass_kernel_spmd(nc, in_maps, core_ids=list(range(N_CORES)))
    out = np.concatenate([r["y"] for r in res.results], axis=0)
    return out.astype(np.float32).reshape(B, CO, H, W)


# revision 23
# speedup vs baseline: 1.0137x; 1.0137x over previous
"""Binary (sign-quantized weight) 3x3 conv, stride 1, pad 1, on 8 trn2 cores.

Problem: x[32,128,56,56] f32, weight[256,128,3,3] f32, bias[256] f32
         y = conv2d(x, sign(weight), pad=1) + bias      -> [32,256,56,56] f32

Strategy (v4, fp8 DoubleRow + per-block chunk tiles):
  - Data-parallel over batch: 4 images per core, weight/bias replicated.
  - x is decomposed into two fp8e4 planes: hi = e4m3(x), lo = e4m3(x-hi).
    One DoubleRow matmul contracts both planes (K = 2x128) against a
    (sign(w), sign(w)) stationary pair at 0.5 cycles/output element --
    2x the bf16 rate at ~bf16 accuracy.
  - Each 8-row output block owns a small chunk tile [128, 2, 572]
    holding its 10 input rows (block + halo) flat at physical width 57:
    one zero pad col per row, shared between row r's right edge and row
    r+1's left edge. A 3x3 tap is then ONE flat shifted-segment matmul
    (junk output col cp=0 per row discarded at drain; vertical padding =
    row-range narrowing, PSUM has_written covers partial writes). Small
    per-block tiles keep Tile's bounding-range dependencies tight so
    matmuls start as soon as their own chunk is packed.
  - DMA: inputs on the Pool queue, weights (split per kh) + outputs on
    the SP queue; img 0 arrives in 7 halo-straddling chunks alternating
    between queues, imgs 1-3 as single transfers. Output staged bf16
    per (image, co-half) and DMA'd in halves (finer on the last image);
    host upcasts to f32.
  - Drains (bias add, PSUM -> SBUF bf16) alternate DVE/ACT; hi-packs on
    ACT, lo-packs alternate DVE/gpsimd.
  - A stream of small zero matmuls bridges the PE from t~0 to the first
    real matmul so the p-state ramp never resets.
"""

import sys

sys.path.insert(0, "/opt/trn_rl_repo")

from contextlib import ExitStack

import numpy as np

B, CI, CO, KK, H, W = 32, 128, 256, 3, 56, 56
N_CORES = 8
B_SH = B // N_CORES  # 4 images per core
WP = W + 1  # physical row width: shared zero pad col
ROWS = 8  # output rows per block
N_RB = H // ROWS  # 7 row blocks
NOUT = ROWS * WP  # 456 <= 512 (one PSUM bank)
SLOTS = ROWS + 2  # input rows per chunk tile (block + halo)
LC = 1 + SLOTS * WP + 1  # chunk tile flat length (front/back guards)
XPC_BUFS = 6

_NC_CACHE = None


def _build():
    import concourse.tile as tile
    from concourse import bacc, mybir

    nc = bacc.Bacc("TRN2", target_bir_lowering=False, debug=False)

    x_d = nc.dram_tensor("x", [B_SH, CI, H, W], mybir.dt.float32, kind="ExternalInput")
    wt_d = nc.dram_tensor(
        "wt", [CI, KK * KK * 2 * CO], mybir.dt.float8e4, kind="ExternalInput"
    )
    b_d = nc.dram_tensor("bias2", [128, CO // 128], mybir.dt.float32, kind="ExternalInput")
    y_d = nc.dram_tensor("y", [B_SH, CO, H * W], mybir.dt.bfloat16, kind="ExternalOutput")

    x_img = x_d.ap().rearrange("b c h w -> b c (h w)")
    y_ap = y_d.ap()

    with tile.TileContext(nc) as tc:
        with ExitStack() as ctx:
            singles = ctx.enter_context(tc.tile_pool(name="singles", bufs=1))
            xpc_pool = ctx.enter_context(tc.tile_pool(name="xpc", bufs=XPC_BUFS))
            ps_pool = ctx.enter_context(
                tc.tile_pool(name="ps", bufs=8, space="PSUM")
            )
            yo_pool = ctx.enter_context(tc.tile_pool(name="yo", bufs=2))

            # [ci, tap, (A,B) slab pair, co] fp8; A and B both sign(w)
            w_bin = singles.tile([CI, KK * KK, 2, CO], mybir.dt.float8e4)
            wt_ap = wt_d.ap().rearrange("p (t two c) -> p t two c", two=2, c=CO)

            # PE warm-up bridge
            N_WARM = 45
            warm_w = singles.tile([128, 128], mybir.dt.bfloat16)
            warm_x = singles.tile([128, 112], mybir.dt.bfloat16)
            nc.vector.memset(warm_w[:, :], 0.0)
            nc.vector.memset(warm_x[:, :], 0.0)
            for _ in range(N_WARM):
                warm_ps = ps_pool.tile([128, 112], mybir.dt.float32, tag="ps")
                nc.tensor.matmul(
                    warm_ps[:, :], warm_w[:, :], warm_x[:, :], start=True, stop=True
                )
            warm_a = singles.tile([128, 1], mybir.dt.float32)
            nc.vector.memset(warm_a[:, :], 0.0)
            nc.scalar.activation(
                warm_a[:, :], warm_a[:, :], mybir.ActivationFunctionType.Identity
            )

            # whole-image f32 staging, double buffered (static so reuse keeps
            # SBUF offsets fixed)
            xfs = [
                singles.tile([CI, H * W], mybir.dt.float32, name=f"xf{i}")
                for i in range(2)
            ]

            n_alloc = 0

            def alloc_chunk():
                """Chunk tile; zero the pad cols once per physical buffer.

                The pool hands buffers out round-robin, packs never write pad
                cols, so zeroing the first XPC_BUFS allocations covers all."""
                nonlocal n_alloc
                t = xpc_pool.tile([CI, 2, LC], mybir.dt.float8e4, tag="xpc")
                if n_alloc < XPC_BUFS:
                    for i in range(2):
                        nc.gpsimd.memset(
                            t[:, i, 1 : 1 + SLOTS * WP].rearrange(
                                "p (h w) -> p h w", w=WP
                            )[:, :, 0:1],
                            0.0,
                        )
                        # back guard doubles as slot 9's right pad
                        nc.gpsimd.memset(t[:, i, LC - 1 : LC], 0.0)
                n_alloc += 1
                return t

            def pack_chunk(xpt, xf, rb, lo_eng, sub=None):
                """hi/lo packs of block rb's input rows into its chunk tile."""
                r0 = rb * ROWS
                ra = max(r0 - 1, 0)  # first source row
                rz = min(r0 + ROWS + 1, H)  # end source row
                if sub is not None:
                    ra, rz = sub
                s0 = ra - (r0 - 1)  # first slot written
                n = rz - ra
                body = lambda i: xpt[
                    :, i, 1 + s0 * WP : 1 + (s0 + n) * WP
                ].rearrange("p (h w) -> p h w", w=WP)[:, :, 1 : 1 + W]
                xfv = xf.rearrange("p (h w) -> p h w", w=W)[:, ra:rz, :]
                hi = body(0)
                nc.scalar.activation(
                    hi, xfv, mybir.ActivationFunctionType.Identity
                )
                lo_eng.tensor_tensor(
                    out=body(1), in0=xfv, in1=hi, op=mybir.AluOpType.subtract
                )

            def wt_dma(kh):
                nc.sync.dma_start(
                    out=w_bin[:, kh * KK : (kh + 1) * KK, :, :],
                    in_=wt_ap[:, kh * KK : (kh + 1) * KK, :, :],
                )

            # startup: img-0 chunk DMAs straddle block halos and alternate
            # Pool/SP queues; weight thirds interleave on SP; bias last
            bias_sb = singles.tile([128, CO // 128], mybir.dt.float32)
            bounds = [0, 9, 17, 25, 33, 41, 49, 56]
            chunk_engs = [
                nc.sync, nc.gpsimd, nc.sync, nc.gpsimd,
                nc.sync, nc.gpsimd, nc.sync,
            ]
            xpcs = {}

            def lo_eng_for(rb):
                return nc.gpsimd if rb % 3 == 2 else nc.vector

            def chunk0_dma(c):
                r0, r1 = bounds[c], bounds[c + 1]
                chunk_engs[c].dma_start(
                    out=xfs[0][:, r0 * W : r1 * W], in_=x_img[0, :, r0 * W : r1 * W]
                )
                xpcs[0, c] = alloc_chunk()
                pack_chunk(xpcs[0, c], xfs[0], c, lo_eng_for(c))

            chunk0_dma(0)
            wt_dma(0)
            chunk0_dma(1)
            chunk0_dma(2)
            wt_dma(1)
            chunk0_dma(3)
            wt_dma(2)
            chunk0_dma(4)
            chunk0_dma(5)
            chunk0_dma(6)
            nc.sync.dma_start(out=bias_sb[:, :], in_=b_d.ap())

            for b in range(B_SH):
                if b > 0:
                    xf = xfs[b % 2]
                    nc.gpsimd.dma_start(out=xf[:, :], in_=x_img[b, :, :])
                    for rb in range(N_RB):
                        xpcs[b, rb] = alloc_chunk()
                        pack_chunk(xpcs[b, rb], xf, rb, lo_eng_for(rb))

                yb = yo_pool.tile(
                    [128, CO // 128, H * W], mybir.dt.bfloat16, tag="yb"
                )
                for rb in range(N_RB):
                    r0 = rb * ROWS
                    xpt = xpcs.pop((b, rb))
                    for c2 in range(CO // 128):
                        ps = ps_pool.tile([128, NOUT], mybir.dt.float32, tag="ps")
                        i = 0
                        for kh in range(KK):
                            # rows (within block) whose input row is in [0, H)
                            a = max(0, (1 - kh) - r0)
                            bb = min(ROWS, (H + 1) - kh - r0)
                            for kw in range(KK):
                                q0 = 1 + (a + kh) * WP + (kw - 1)
                                nrow = bb - a
                                assert 0 <= q0 and q0 + nrow * WP <= LC
                                nc.tensor.matmul(
                                    ps[:, a * WP : bb * WP],
                                    w_bin[:, kh * KK + kw, :, c2 * 128 : (c2 + 1) * 128],
                                    xpt[:, :, q0 : q0 + nrow * WP],
                                    start=(i == 0),
                                    stop=(i == KK * KK - 1),
                                    perf_mode=mybir.MatmulPerfMode.DoubleRow,
                                    skip_group_check=True,
                                )
                                i += 1
                        ys = yb[:, c2, rb * ROWS * W : (rb + 1) * ROWS * W]
                        ps3 = ps.rearrange("p (r w) -> p r w", w=WP)
                        if (rb + c2) % 2 == 0:
                            nc.vector.tensor_scalar_add(
                                ys.rearrange("p (r w) -> p r w", w=W),
                                ps3[:, :, 1 : 1 + W],
                                bias_sb[:, c2 : c2 + 1],
                            )
                        else:
                            nc.scalar.activation(
                                ys.rearrange("p (r w) -> p r w", w=W),
                                ps3[:, :, 1 : 1 + W],
                                mybir.ActivationFunctionType.Identity,
                                bias=bias_sb[:, c2 : c2 + 1],
                            )
                    # output DMAs: rb 0-3 as one transfer per (b,c2); the rest
                    # per-rb on the last image (short tail) or as one transfer
                    if rb == 3:
                        for c2 in range(CO // 128):
                            nc.sync.dma_start(
                                out=y_ap[b, c2 * 128 : (c2 + 1) * 128, : 4 * ROWS * W],
                                in_=yb[:, c2, : 4 * ROWS * W],
                            )
                    elif rb == N_RB - 1:
                        if b == B_SH - 1:
                            # keep the final (latest-draining) transfers on the
                            # fast HWDGE queue; push earlier ones to Pool
                            for r2 in range(4, N_RB):
                                for c2 in range(CO // 128):
                                    eng = nc.gpsimd if (c2 == 1 and r2 < 6) else nc.sync
                                    eng.dma_start(
                                        out=y_ap[
                                            b,
                                            c2 * 128 : (c2 + 1) * 128,
                                            r2 * ROWS * W : (r2 + 1) * ROWS * W,
                                        ],
                                        in_=yb[:, c2, r2 * ROWS * W : (r2 + 1) * ROWS * W],
                                    )
                        else:
                            for c2 in range(CO // 128):
                                nc.sync.dma_start(
                                    out=y_ap[b, c2 * 128 : (c2 + 1) * 128, 4 * ROWS * W :],
                                    in_=yb[:, c2, 4 * ROWS * W :],
                                )
    nc.compile()
    return nc


def _get_nc():
    global _NC_CACHE
    if _NC_CACHE is None:
        _NC_CACHE = _build()
    return _NC_CACHE


def kernel(x, weight, bias):
    from concourse.bass_utils import run_bass_kernel_spmd

    import ml_dtypes

    x = np.ascontiguousarray(np.asarray(x, dtype=np.float32))
    weight = np.asarray(weight, dtype=np.float32)
    bias = np.asarray(bias, dtype=np.float32)

    # binarize on host; {-1,0,1} is exact in fp8e4. [co,ci,kh,kw] ->
    # [ci, (kh kw), co], duplicated into (A,B) slab pairs for DoubleRow.
    ws = np.sign(weight).transpose(1, 2, 3, 0).reshape(CI, KK * KK, 1, CO)
    wt = np.ascontiguousarray(
        np.broadcast_to(ws, (CI, KK * KK, 2, CO)).reshape(CI, KK * KK * 2 * CO)
    ).astype(ml_dtypes.float8_e4m3fn)
    # bias2[p, c2] = bias[c2*128 + p]
    bias2 = np.ascontiguousarray(bias.reshape(CO // 128, 128).T)

    nc = _get_nc()
    in_maps = [
        {"x": x[i * B_SH : (i + 1) * B_SH], "wt": wt, "bias2": bias2}
        for i in range(N_CORES)
    ]
    res = run_bass_kernel_spmd(nc, in_maps, core_ids=list(range(N_CORES)))
    out = np.concatenate([r["y"] for r in res.results], axis=0)
    return out.astype(np.float32).reshape(B, CO, H, W)
